# revision 9
# baseline (speedup 1.0000x reference)
"""Trainium2 Bass kernel for sliding-window multi-head attention
(nn_MultiHeadAttention_55181739819367).

Sharding: sequence-parallel x batch-parallel. 8 cores, each owns 512
query rows of one batch element (B=2 x 4 sequence chunks). K/V halo of
n_window=256 rows on each side is shipped with each chunk (zero-padded
at sequence edges), so cores are fully independent (no collectives).

Per-core pipeline (fp32 data; matmuls in float32r for full PE rate):
  1. PE-transpose input chunks to get X^T layouts for the projections.
  2. QKV projections: q^T [hd, 512] (pre-scaled by 1/sqrt(dk)),
     k^T [hd, 1024], v [1024, hd].
  3. Banded attention per (head, 256-query-block): scores for the
     768-wide key stripe via PE (plus additive -1e30 band mask applied
     with identity-matmul PSUM accumulation), exp on ACT with fused
     per-row accumulation (softmax denominator), reciprocal on DVE,
     normalization on GPSIMD, PE-transpose of the normalized attn for
     the ctx matmul, ctx^T accumulation on PE (col-group packed pairs).
  4. Output projection (+bias via ones-row matmul), residual add,
     LayerNorm (bn_stats/bn_aggr), y out.
Outputs: y rows [512, 1024] and the 768-wide attention stripes; the
host pastes stripes into the full [B, H, S, S] attention tensor (exact
zeros elsewhere).
"""

import sys
import math

import numpy as np

sys.path.insert(0, "/opt/trn_rl_repo")

import concourse.bass as bass  # noqa: E402
import concourse.tile as tile  # noqa: E402
from concourse import mybir  # noqa: E402
from concourse.vector_clock import ScopedClock  # noqa: E402

F32 = mybir.dt.float32
F32R = mybir.dt.float32r

# Problem dims (hardcoded per contest contract)
B = 2
S = 2048
D = 1024
H = 16
DK = 64
DV = 64
HD = H * DK  # 1024
NW = 256
LN_EPS = 1e-5
N_CORES = 8

SEQ_SHARDS = 4
S_CORE = S // SEQ_SHARDS          # 512 query rows per core
HALO = S_CORE + 2 * NW            # 1024 k/v rows per core
NQB = S_CORE // 256               # 256-row query blocks per core (2)
NSB = S_CORE // 128               # 128-row query sub-blocks (4)
STRIPE = 3 * NW                   # 768 key columns per query block
SJC = STRIPE // 128               # 6 key chunks per stripe
DC = D // 128                     # 8 contraction chunks over model dim
MC = HD // 128                    # 8 chunks over head*dk dim
JC = HALO // 128                  # 8 key row chunks
HT = H // 2                       # head-pair tiles (128 partitions)
MASK_NEG = -1.0e30


class SafeTileContext(tile.TileContext):
    """TileContext that never emits more than one sync-wait command per
    instruction: this walrus build rejects multi-wait instructions
    ("Too many sync wait commands"), so excess waits are peeled off onto
    same-engine NoOps committed immediately before the instruction."""

    _wsplit = 0

    def _commit_instruction(self, inst, lazy_reg_writes=True):
        si = getattr(inst, "sync_info", None)
        if (si is not None and si.on_wait and len(si.on_wait) > 1
                and inst.engine != mybir.EngineType.Unassigned):
            waits = list(si.on_wait)
            inst.sync_info = mybir.SyncInfo(
                on_wait=[waits[-1]], on_update=list(si.on_update))
            for w in waits[:-1]:
                SafeTileContext._wsplit += 1
                nop = mybir.InstNoOp(name=f"wsplit-{SafeTileContext._wsplit}")
                nop.engine = inst.engine
                nop.sync_info = mybir.SyncInfo(on_wait=[w], on_update=[])
                super()._commit_instruction(nop, lazy_reg_writes=False)
        super()._commit_instruction(inst, lazy_reg_writes)

    def _drain_and_barrier(self, tick_clock, wait_clock):
        drain_inst = self.nc.sync.drain()
        wait_clock.add_sem_waits(
            drain_inst.ins, ScopedClock({None: tick_clock.global_clock})
        )
        si = drain_inst.ins.sync_info
        waits = list(si.on_wait) if si else []
        if len(waits) > 1:
            drain_inst.ins.sync_info = mybir.SyncInfo(
                on_wait=waits[:1], on_update=list(si.on_update)
            )
            for w in waits[1:]:
                nop = self.nc.sync.nop(nofuse=True, hint="drain_wait_split")
                nop.ins.sync_info = mybir.SyncInfo(on_wait=[w], on_update=[])

        self.nc.all_engine_barrier()
        assert self.sems is not None
        popped = self.nc._tile_sem_poison_stack.pop()
        assert popped is self._sem_poison
        self.nc.clear_and_free_semaphores(list(self.sems.allocated().values()))
        self.nc.all_engine_barrier()


def _bcast_rows(ap, p):
    """[N] dram AP -> [p, N] with partition step 0 (row broadcast)."""
    return bass.AP(tensor=ap.tensor, offset=ap.offset,
                   ap=[[0, p]] + list(ap.ap))


def build_program():
    from contextlib import ExitStack

    nc = bass.Bass("TRN2", target_bir_lowering=False, debug=False,
                   num_devices=N_CORES)

    # fp32r tensors carry plain IEEE fp32 bits; the label lets the PE run
    # its single-pass reduced-precision fp32 matmul path.
    xq_d = nc.dram_tensor("xq", [S_CORE, D], F32R, kind="ExternalInput")
    xk_d = nc.dram_tensor("xk", [HALO, D], F32R, kind="ExternalInput")
    xv_d = nc.dram_tensor("xv", [HALO, D], F32R, kind="ExternalInput")
    wq_d = nc.dram_tensor("wq", [D, HD], F32R, kind="ExternalInput")
    wk_d = nc.dram_tensor("wk", [D, HD], F32R, kind="ExternalInput")
    wv_d = nc.dram_tensor("wv", [D, HD], F32R, kind="ExternalInput")
    wo_d = nc.dram_tensor("wo", [HD, D], F32R, kind="ExternalInput")
    bq8_d = nc.dram_tensor("bq8", [HD], F32, kind="ExternalInput")
    bk_d = nc.dram_tensor("bk", [HD], F32, kind="ExternalInput")
    bv_d = nc.dram_tensor("bv", [HD], F32R, kind="ExternalInput")
    bo_d = nc.dram_tensor("bo", [D], F32R, kind="ExternalInput")
    gamma_d = nc.dram_tensor("gamma", [D], F32, kind="ExternalInput")
    beta_d = nc.dram_tensor("beta", [D], F32, kind="ExternalInput")
    mask_d = nc.dram_tensor("mask", [NQB, 2, 128, STRIPE], F32R,
                            kind="ExternalInput")
    ident_d = nc.dram_tensor("ident", [128, 128], F32R, kind="ExternalInput")
    ones_d = nc.dram_tensor("ones1", [1, 128], F32R, kind="ExternalInput")
    y_d = nc.dram_tensor("y", [S_CORE, D], F32, kind="ExternalOutput")
    attn_d = nc.dram_tensor("attn", [H, NQB, 2, 128, STRIPE], F32R,
                            kind="ExternalOutput")

    with SafeTileContext(nc) as tc, ExitStack() as top:
        singles = top.enter_context(tc.tile_pool(name="singles", bufs=1))
        ident = singles.tile([128, 128], F32R)
        nc.sync.dma_start(out=ident, in_=ident_d.ap())
        ones1 = singles.tile([1, 128], F32R)
        nc.sync.dma_start(out=ones1, in_=ones_d.ap())
        eps_t = singles.tile([128, 1], F32)
        nc.vector.memset(eps_t, LN_EPS)
        bq8_t = singles.tile([128, MC], F32)
        nc.sync.dma_start(out=bq8_t, in_=bq8_d.ap().rearrange("(c p) -> p c", p=128))
        bk_t = singles.tile([128, MC], F32)
        nc.sync.dma_start(out=bk_t, in_=bk_d.ap().rearrange("(c p) -> p c", p=128))
        bv_row = singles.tile([1, HD], F32R)
        nc.sync.dma_start(out=bv_row, in_=bv_d.ap().rearrange("(o d) -> o d", o=1))
        bo_row = singles.tile([1, D], F32R)
        nc.sync.dma_start(out=bo_row, in_=bo_d.ap().rearrange("(o d) -> o d", o=1))
        gamma_bc = singles.tile([128, D], F32)
        nc.sync.dma_start(out=gamma_bc, in_=_bcast_rows(gamma_d.ap(), 128))
        beta_bc = singles.tile([128, D], F32)
        nc.sync.dma_start(out=beta_bc, in_=_bcast_rows(beta_d.ap(), 128))

        aux_ps = top.enter_context(
            tc.tile_pool(name="aux_ps", bufs=3, space="PSUM"))

        # Persistent activations
        xq_nat_p = top.enter_context(tc.tile_pool(name="xq_nat", bufs=NSB))
        qT_p = top.enter_context(tc.tile_pool(name="qT", bufs=MC))
        kT_p = top.enter_context(tc.tile_pool(name="kT", bufs=MC))
        v_p = top.enter_context(tc.tile_pool(name="v", bufs=JC))
        ctxT_p = top.enter_context(tc.tile_pool(name="ctxT", bufs=MC))

        xq_nat = [xq_nat_p.tile([128, D], F32R, tag="xq_nat", name="xq_nat")
                  for _ in range(NSB)]
        qT = [qT_p.tile([128, S_CORE], F32R, tag="qT", name="qT")
              for _ in range(MC)]
        kT = [kT_p.tile([128, HALO], F32R, tag="kT", name="kT")
              for _ in range(MC)]
        v_sb = [v_p.tile([128, HD], F32R, tag="v", name="v")
                for _ in range(JC)]
        ctxT = [ctxT_p.tile([128, S_CORE], F32R, tag="ctxT", name="ctxT")
                for _ in range(HT)]

        def transpose_in(x_dram, rows, xt_pool, nat_pool, keep_tiles=None):
            """Load [rows, D] natural, return X^T as DC tiles [128, rows]."""
            xt = [xt_pool.tile([128, rows], F32R, tag="xt", name="xt")
                  for _ in range(DC)]
            for rr in range(rows // 128):
                if keep_tiles is not None:
                    nat = keep_tiles[rr]
                else:
                    nat = nat_pool.tile([128, D], F32R, tag="nat", name="nat")
                nc.sync.dma_start(out=nat,
                                  in_=x_dram.ap()[rr * 128:(rr + 1) * 128, :])
                for c in range(DC):
                    pt = aux_ps.tile([128, 128], F32R, tag="aux", name="aux")
                    nc.tensor.transpose(pt, nat[:, c * 128:(c + 1) * 128],
                                        ident)
                    nc.scalar.copy(xt[c][:, rr * 128:(rr + 1) * 128], pt)
            return xt

        # ---- Phase P: projections (k, v, then q) ----
        with ExitStack() as pp:
            big_ps = pp.enter_context(
                tc.tile_pool(name="big_ps", bufs=2, space="PSUM"))
            nat_p = pp.enter_context(tc.tile_pool(name="nat", bufs=3))

            # k^T
            with ExitStack() as ps_scope:
                xt_p = ps_scope.enter_context(tc.tile_pool(name="xkt", bufs=DC))
                w_p = ps_scope.enter_context(tc.tile_pool(name="wk", bufs=DC))
                xkT = transpose_in(xk_d, HALO, xt_p, nat_p)
                wk_sb = [w_p.tile([128, HD], F32R, tag="w", name="w")
                         for _ in range(DC)]
                for c in range(DC):
                    nc.sync.dma_start(out=wk_sb[c],
                                      in_=wk_d.ap()[c * 128:(c + 1) * 128, :])
                for m in range(MC):
                    for hf in range(HALO // 512):
                        ps_t = big_ps.tile([128, 512], F32, tag="big",
                                           name="big")
                        for kc in range(DC):
                            nc.tensor.matmul(
                                ps_t,
                                lhsT=wk_sb[kc][:, m * 128:(m + 1) * 128],
                                rhs=xkT[kc][:, hf * 512:(hf + 1) * 512],
                                start=(kc == 0), stop=(kc == DC - 1))
                        nc.scalar.activation(
                            kT[m][:, hf * 512:(hf + 1) * 512], ps_t,
                            mybir.ActivationFunctionType.Identity,
                            bias=bk_t[:, m:m + 1], scale=1.0)

            # v
            with ExitStack() as ps_scope:
                xt_p = ps_scope.enter_context(tc.tile_pool(name="xvt", bufs=DC))
                w_p = ps_scope.enter_context(tc.tile_pool(name="wv", bufs=DC))
                xvT = transpose_in(xv_d, HALO, xt_p, nat_p)
                wv_sb = [w_p.tile([128, HD], F32R, tag="w", name="w")
                         for _ in range(DC)]
                for c in range(DC):
                    nc.sync.dma_start(out=wv_sb[c],
                                      in_=wv_d.ap()[c * 128:(c + 1) * 128, :])
                for jr in range(JC):
                    for hf in range(HD // 512):
                        ps_t = big_ps.tile([128, 512], F32, tag="big",
                                           name="big")
                        for kc in range(DC):
                            nc.tensor.matmul(
                                ps_t,
                                lhsT=xvT[kc][:, jr * 128:(jr + 1) * 128],
                                rhs=wv_sb[kc][:, hf * 512:(hf + 1) * 512],
                                start=(kc == 0), stop=False)
                        nc.tensor.matmul(
                            ps_t, lhsT=ones1,
                            rhs=bv_row[:, hf * 512:(hf + 1) * 512],
                            start=False, stop=True)
                        nc.scalar.copy(v_sb[jr][:, hf * 512:(hf + 1) * 512],
                                       ps_t)

            # q^T (pre-scaled by 1/sqrt(DK); bias bq8 = bq/sqrt(DK) host-side)
            with ExitStack() as ps_scope:
                xt_p = ps_scope.enter_context(tc.tile_pool(name="xqt", bufs=DC))
                w_p = ps_scope.enter_context(tc.tile_pool(name="wq", bufs=DC))
                xqT = transpose_in(xq_d, S_CORE, xt_p, nat_p,
                                   keep_tiles=xq_nat)
                wq_sb = [w_p.tile([128, HD], F32R, tag="w", name="w")
                         for _ in range(DC)]
                for c in range(DC):
                    nc.sync.dma_start(out=wq_sb[c],
                                      in_=wq_d.ap()[c * 128:(c + 1) * 128, :])
                for m in range(MC):
                    ps_t = big_ps.tile([128, S_CORE], F32, tag="big",
                                       name="big")
                    for kc in range(DC):
                        nc.tensor.matmul(
                            ps_t,
                            lhsT=wq_sb[kc][:, m * 128:(m + 1) * 128],
                            rhs=xqT[kc],
                            start=(kc == 0), stop=(kc == DC - 1))
                    nc.scalar.activation(
                        qT[m], ps_t,
                        mybir.ActivationFunctionType.Identity,
                        bias=bq8_t[:, m:m + 1], scale=1.0 / math.sqrt(DK))

        # ---- Phase A: banded attention ----
        with ExitStack() as pa:
            score_ps = pa.enter_context(
                tc.tile_pool(name="score_ps", bufs=2, space="PSUM"))
            mask_p = pa.enter_context(tc.tile_pool(name="mask", bufs=2 * NQB))
            exp_p = pa.enter_context(tc.tile_pool(name="exp", bufs=3))
            at_p = pa.enter_context(tc.tile_pool(name="at", bufs=4))
            expT_p = pa.enter_context(
                tc.tile_pool(name="expT", bufs=2 * SJC + 4))
            den_p = pa.enter_context(tc.tile_pool(name="den", bufs=6))

            mask_sb = [[mask_p.tile([128, STRIPE], F32R, tag="mask",
                                    name="mask")
                        for _ in range(2)] for _ in range(NQB)]
            for t in range(NQB):
                for o in range(2):
                    nc.sync.dma_start(out=mask_sb[t][o],
                                      in_=mask_d.ap()[t, o])

            for mt in range(HT):
                for t in range(NQB):
                    j0 = t * 256
                    pair_expT = []
                    for hh in range(2):
                        h = 2 * mt + hh
                        po = hh * DK
                        expT = [expT_p.tile([128, 256], F32R, tag="expT",
                                            name="expT")
                                for _ in range(SJC)]
                        pair_expT.append(expT)
                        for o in range(2):
                            i_off = t * 256 + o * 128
                            ps_s = score_ps.tile([128, STRIPE], F32,
                                                 tag="score", name="score")
                            lhs = qT[mt][po:po + DK, i_off:i_off + 128]
                            nc.tensor.matmul(
                                ps_s[:, 0:512], lhsT=lhs,
                                rhs=kT[mt][po:po + DK, j0:j0 + 512],
                                start=True, stop=False)
                            nc.tensor.matmul(
                                ps_s[:, 0:512], lhsT=ident,
                                rhs=mask_sb[t][o][:, 0:512],
                                start=False, stop=True)
                            nc.tensor.matmul(
                                ps_s[:, 512:STRIPE], lhsT=lhs,
                                rhs=kT[mt][po:po + DK, j0 + 512:j0 + STRIPE],
                                start=True, stop=False)
                            nc.tensor.matmul(
                                ps_s[:, 512:STRIPE], lhsT=ident,
                                rhs=mask_sb[t][o][:, 512:STRIPE],
                                start=False, stop=True)
                            exp_t = exp_p.tile([128, STRIPE], F32, tag="exp",
                                               name="exp")
                            den = den_p.tile([128, 1], F32, tag="den",
                                             name="den")
                            nc.scalar.activation(
                                exp_t, ps_s,
                                mybir.ActivationFunctionType.Exp,
                                accum_out=den)
                            rec = den_p.tile([128, 1], F32, tag="rec",
                                             name="rec")
                            nc.vector.reciprocal(rec, den)
                            at = at_p.tile([128, STRIPE], F32R, tag="at",
                                           name="at")
                            nc.gpsimd.tensor_scalar_mul(at, exp_t, rec)
                            nc.sync.dma_start(out=attn_d.ap()[h, t, o],
                                              in_=at)
                            for jc in range(SJC):
                                pt = aux_ps.tile([128, 128], F32R, tag="aux",
                                                 name="aux")
                                nc.tensor.transpose(
                                    pt, at[:, jc * 128:(jc + 1) * 128], ident)
                                nc.vector.tensor_copy(
                                    expT[jc][:, o * 128:(o + 1) * 128], pt)
                    # ctx per head; odd head lands in ctxT partitions 64:128
                    # via an SBUF->SBUF DMA (engines can't shift partitions)
                    for hh in range(2):
                        h = 2 * mt + hh
                        ps_c = aux_ps.tile([64, 256], F32, tag="aux",
                                           name="aux")
                        for jc in range(SJC):
                            jr = 2 * t + jc
                            nc.tensor.matmul(
                                ps_c,
                                lhsT=v_sb[jr][:, h * DV:(h + 1) * DV],
                                rhs=pair_expT[hh][jc],
                                start=(jc == 0), stop=(jc == SJC - 1))
                        if hh == 0:
                            nc.scalar.copy(
                                ctxT[mt][0:64, t * 256:(t + 1) * 256], ps_c)
                        else:
                            stage = at_p.tile([64, 256], F32R, tag="stage",
                                              name="stage")
                            nc.scalar.copy(stage, ps_c)
                            nc.sync.dma_start(
                                out=ctxT[mt][64:128, t * 256:(t + 1) * 256],
                                in_=stage)

        # ---- Phase O: output projection + residual + LayerNorm ----
        with ExitStack() as po_:
            big_ps = po_.enter_context(
                tc.tile_pool(name="big_ps2", bufs=2, space="PSUM"))
            wo_p = po_.enter_context(tc.tile_pool(name="wo", bufs=MC))
            x_p = po_.enter_context(tc.tile_pool(name="x", bufs=NSB))
            y_p = po_.enter_context(tc.tile_pool(name="y", bufs=2))
            st_p = po_.enter_context(tc.tile_pool(name="st", bufs=8))

            wo_sb = [wo_p.tile([128, D], F32R, tag="wo", name="wo")
                     for _ in range(MC)]
            for c in range(MC):
                nc.sync.dma_start(out=wo_sb[c],
                                  in_=wo_d.ap()[c * 128:(c + 1) * 128, :])

            for s_i in range(NSB):
                x_sb = x_p.tile([128, D], F32, tag="x", name="x")
                for hf in range(D // 512):
                    ps_t = big_ps.tile([128, 512], F32, tag="big2",
                                       name="big2")
                    for kc in range(MC):
                        nc.tensor.matmul(
                            ps_t,
                            lhsT=ctxT[kc][:, s_i * 128:(s_i + 1) * 128],
                            rhs=wo_sb[kc][:, hf * 512:(hf + 1) * 512],
                            start=(kc == 0), stop=False)
                    nc.tensor.matmul(
                        ps_t, lhsT=ones1,
                        rhs=bo_row[:, hf * 512:(hf + 1) * 512],
                        start=False, stop=True)
                    nc.scalar.copy(x_sb[:, hf * 512:(hf + 1) * 512], ps_t)
                nc.vector.tensor_add(x_sb, x_sb, xq_nat[s_i])
                # LayerNorm over D
                stats = st_p.tile([128, D // 512, 6], F32, tag="stats",
                                  name="stats")
                for g in range(D // 512):
                    nc.vector.bn_stats(stats[:, g, :],
                                       x_sb[:, g * 512:(g + 1) * 512])
                mv = st_p.tile([128, 2], F32, tag="mv", name="mv")
                nc.vector.bn_aggr(mv, stats)
                std = st_p.tile([128, 1], F32, tag="std", name="std")
                nc.scalar.activation(std, mv[:, 1:2],
                                     mybir.ActivationFunctionType.Sqrt,
                                     bias=eps_t, scale=1.0)
                rstd = st_p.tile([128, 1], F32, tag="rstd", name="rstd")
                nc.vector.reciprocal(rstd, std)
                y_sb = y_p.tile([128, D], F32, tag="y", name="y")
                nc.vector.tensor_scalar(
                    out=y_sb, in0=x_sb,
                    scalar1=mv[:, 0:1], scalar2=rstd,
                    op0=mybir.AluOpType.subtract, op1=mybir.AluOpType.mult)
                nc.vector.tensor_mul(y_sb, y_sb, gamma_bc)
                nc.vector.tensor_add(y_sb, y_sb, beta_bc)
                nc.sync.dma_start(out=y_d.ap()[s_i * 128:(s_i + 1) * 128, :],
                                  in_=y_sb)

    return nc


_PROGRAM = None


def _get_program():
    global _PROGRAM
    if _PROGRAM is None:
        _PROGRAM = build_program()
    return _PROGRAM


def _make_core_inputs(Q, K, V, Wq, bq, Wk, bk, Wv, bv, Wo, bo, gamma, beta):
    scale = 1.0 / math.sqrt(DK)
    ident_np = np.eye(128, dtype=np.float32)
    ones_np = np.ones((1, 128), np.float32)
    in_maps = []
    for c in range(N_CORES):
        b = c // SEQ_SHARDS
        q0 = (c % SEQ_SHARDS) * S_CORE
        lo = q0 - NW
        hi = q0 + S_CORE + NW
        clo, chi = max(lo, 0), min(hi, S)
        xk = np.zeros((HALO, D), np.float32)
        xv = np.zeros((HALO, D), np.float32)
        xk[clo - lo:chi - lo] = K[b, clo:chi]
        xv[clo - lo:chi - lo] = V[b, clo:chi]
        mask = np.full((NQB, 2, 128, STRIPE), MASK_NEG, np.float32)
        for t in range(NQB):
            for o in range(2):
                i_idx = q0 + t * 256 + o * 128 + np.arange(128)[:, None]
                j_idx = q0 + t * 256 - NW + np.arange(STRIPE)[None, :]
                ok = (np.abs(i_idx - j_idx) < NW) & (j_idx >= 0) & (j_idx < S)
                mask[t, o][ok] = 0.0
        in_maps.append({
            "xq": np.ascontiguousarray(Q[b, q0:q0 + S_CORE]).astype(np.float32),
            "xk": xk, "xv": xv,
            "wq": np.asarray(Wq, np.float32), "wk": np.asarray(Wk, np.float32),
            "wv": np.asarray(Wv, np.float32), "wo": np.asarray(Wo, np.float32),
            "bq8": (np.asarray(bq, np.float32) * scale).astype(np.float32),
            "bk": np.asarray(bk, np.float32), "bv": np.asarray(bv, np.float32),
            "bo": np.asarray(bo, np.float32),
            "gamma": np.asarray(gamma, np.float32),
            "beta": np.asarray(beta, np.float32),
            "mask": mask,
            "ident": ident_np, "ones1": ones_np,
        })
    return in_maps


def _assemble(results):
    y = np.zeros((B, S, D), np.float32)
    attn = np.zeros((B, H, S, S), np.float32)
    for c in range(N_CORES):
        b = c // SEQ_SHARDS
        q0 = (c % SEQ_SHARDS) * S_CORE
        y[b, q0:q0 + S_CORE] = results[c]["y"]
        stripes = np.asarray(results[c]["attn"])  # [H, NQB, 2, 128, STRIPE]
        for t in range(NQB):
            jlo = q0 + t * 256 - NW
            a, bnd = max(jlo, 0), min(jlo + STRIPE, S)
            rows = stripes[:, t].reshape(H, 256, STRIPE)
            i0 = q0 + t * 256
            attn[b, :, i0:i0 + 256, a:bnd] = rows[:, :, a - jlo:bnd - jlo]
    return y, attn


def kernel(Q, K, V, Wq, bq, Wk, bk, Wv, bv, Wo, bo, gamma, beta):
    from concourse.bass_utils import run_bass_kernel_spmd

    nc = _get_program()
    in_maps = _make_core_inputs(Q, K, V, Wq, bq, Wk, bk, Wv, bv, Wo, bo,
                                gamma, beta)
    res = run_bass_kernel_spmd(nc, in_maps, list(range(N_CORES)))
    return _assemble(res.results)


# revision 11
# speedup vs baseline: 2.2858x; 2.2858x over previous
"""Trainium2 Bass kernel for sliding-window multi-head attention
(nn_MultiHeadAttention_55181739819367).

Sharding: sequence-parallel x batch-parallel. 8 cores, each owns 512
query rows of one batch element (B=2 x 4 sequence chunks). K/V halo of
n_window=256 rows on each side is shipped with each chunk (zero-padded
at sequence edges), so cores are fully independent (no collectives).

Per-core pipeline (fp32 data; matmuls in float32r for full PE rate):
  1. PE-transpose input chunks to get X^T layouts for the projections.
  2. QKV projections: q^T [hd, 512] (pre-scaled by 1/sqrt(dk)),
     k^T [hd, 1024], v [1024, hd].
  3. Banded attention per (head, 256-query-block): scores for the
     768-wide key stripe via PE (plus additive -1e30 band mask applied
     with identity-matmul PSUM accumulation), exp on ACT with fused
     per-row accumulation (softmax denominator), reciprocal on DVE,
     normalization on GPSIMD, PE-transpose of the normalized attn for
     the ctx matmul, ctx^T accumulation on PE (col-group packed pairs).
  4. Output projection (+bias via ones-row matmul), residual add,
     LayerNorm (bn_stats/bn_aggr), y out.
Outputs: y rows [512, 1024] and the 768-wide attention stripes; the
host pastes stripes into the full [B, H, S, S] attention tensor (exact
zeros elsewhere).
"""

import sys
import math

import numpy as np

sys.path.insert(0, "/opt/trn_rl_repo")

import concourse.bass as bass  # noqa: E402
import concourse.tile as tile  # noqa: E402
from concourse import mybir  # noqa: E402
from concourse.vector_clock import ScopedClock  # noqa: E402

F32 = mybir.dt.float32
F32R = mybir.dt.float32r

# Problem dims (hardcoded per contest contract)
B = 2
S = 2048
D = 1024
H = 16
DK = 64
DV = 64
HD = H * DK  # 1024
NW = 256
LN_EPS = 1e-5
N_CORES = 8

SEQ_SHARDS = 4
S_CORE = S // SEQ_SHARDS          # 512 query rows per core
HALO = S_CORE + 2 * NW            # 1024 k/v rows per core
NQB = S_CORE // 256               # 256-row query blocks per core (2)
NSB = S_CORE // 128               # 128-row query sub-blocks (4)
STRIPE = 3 * NW                   # 768 key columns per query block
SJC = STRIPE // 128               # 6 key chunks per stripe
DC = D // 128                     # 8 contraction chunks over model dim
MC = HD // 128                    # 8 chunks over head*dk dim
JC = HALO // 128                  # 8 key row chunks
HT = H // 2                       # head-pair tiles (128 partitions)
MASK_NEG = -1.0e30


class SafeTileContext(tile.TileContext):
    """TileContext that never emits more than one sync-wait command per
    instruction: this walrus build rejects multi-wait instructions
    ("Too many sync wait commands"), so excess waits are peeled off onto
    same-engine NoOps committed immediately before the instruction."""

    _wsplit = 0

    def _commit_instruction(self, inst, lazy_reg_writes=True):
        si = getattr(inst, "sync_info", None)
        if (si is not None and si.on_wait and len(si.on_wait) > 1
                and inst.engine != mybir.EngineType.Unassigned):
            waits = list(si.on_wait)
            inst.sync_info = mybir.SyncInfo(
                on_wait=[waits[-1]], on_update=list(si.on_update))
            for w in waits[:-1]:
                SafeTileContext._wsplit += 1
                nop = mybir.InstNoOp(name=f"wsplit-{SafeTileContext._wsplit}")
                nop.engine = inst.engine
                nop.sync_info = mybir.SyncInfo(on_wait=[w], on_update=[])
                super()._commit_instruction(nop, lazy_reg_writes=False)
        super()._commit_instruction(inst, lazy_reg_writes)

    def _drain_and_barrier(self, tick_clock, wait_clock):
        drain_inst = self.nc.sync.drain()
        wait_clock.add_sem_waits(
            drain_inst.ins, ScopedClock({None: tick_clock.global_clock})
        )
        si = drain_inst.ins.sync_info
        waits = list(si.on_wait) if si else []
        if len(waits) > 1:
            drain_inst.ins.sync_info = mybir.SyncInfo(
                on_wait=waits[:1], on_update=list(si.on_update)
            )
            for w in waits[1:]:
                nop = self.nc.sync.nop(nofuse=True, hint="drain_wait_split")
                nop.ins.sync_info = mybir.SyncInfo(on_wait=[w], on_update=[])

        self.nc.all_engine_barrier()
        assert self.sems is not None
        popped = self.nc._tile_sem_poison_stack.pop()
        assert popped is self._sem_poison
        self.nc.clear_and_free_semaphores(list(self.sems.allocated().values()))
        self.nc.all_engine_barrier()


def _bcast_rows(ap, p):
    """[N] dram AP -> [p, N] with partition step 0 (row broadcast)."""
    return bass.AP(tensor=ap.tensor, offset=ap.offset,
                   ap=[[0, p]] + list(ap.ap))


def build_program():
    from contextlib import ExitStack

    nc = bass.Bass("TRN2", target_bir_lowering=False, debug=False,
                   num_devices=N_CORES)

    # fp32r tensors carry plain IEEE fp32 bits; the label lets the PE run
    # its single-pass reduced-precision fp32 matmul path.
    xq_d = nc.dram_tensor("xq", [S_CORE, D], F32R, kind="ExternalInput")
    xk_d = nc.dram_tensor("xk", [HALO, D], F32R, kind="ExternalInput")
    xv_d = nc.dram_tensor("xv", [HALO, D], F32R, kind="ExternalInput")
    wq_d = nc.dram_tensor("wq", [D, HD], F32R, kind="ExternalInput")
    wk_d = nc.dram_tensor("wk", [D, HD], F32R, kind="ExternalInput")
    wv_d = nc.dram_tensor("wv", [D, HD], F32R, kind="ExternalInput")
    wo_d = nc.dram_tensor("wo", [HD, D], F32R, kind="ExternalInput")
    bq8_d = nc.dram_tensor("bq8", [HD], F32, kind="ExternalInput")
    bk_d = nc.dram_tensor("bk", [HD], F32, kind="ExternalInput")
    bv_d = nc.dram_tensor("bv", [HD], F32R, kind="ExternalInput")
    bo_d = nc.dram_tensor("bo", [D], F32R, kind="ExternalInput")
    gamma_d = nc.dram_tensor("gamma", [D], F32, kind="ExternalInput")
    beta_d = nc.dram_tensor("beta", [D], F32, kind="ExternalInput")
    mask_d = nc.dram_tensor("mask", [NQB, 2, 128, STRIPE], F32R,
                            kind="ExternalInput")
    ident_d = nc.dram_tensor("ident", [128, 128], F32R, kind="ExternalInput")
    zeros_d = nc.dram_tensor("zeros128", [128, 128], F32R,
                             kind="ExternalInput")
    ones_d = nc.dram_tensor("ones1", [1, 128], F32R, kind="ExternalInput")
    y_d = nc.dram_tensor("y", [S_CORE, D], F32, kind="ExternalOutput")
    attn_d = nc.dram_tensor("attn", [H, NQB, 2, 128, STRIPE], F32R,
                            kind="ExternalOutput")

    with SafeTileContext(nc) as tc, ExitStack() as top:
        singles = top.enter_context(tc.tile_pool(name="singles", bufs=1))
        ident = singles.tile([128, 128], F32R)
        nc.sync.dma_start(out=ident, in_=ident_d.ap())
        ones1 = singles.tile([1, 128], F32R)
        nc.sync.dma_start(out=ones1, in_=ones_d.ap())
        eps_t = singles.tile([128, 1], F32)
        nc.vector.memset(eps_t, LN_EPS)
        bq8_t = singles.tile([128, MC], F32)
        nc.sync.dma_start(out=bq8_t, in_=bq8_d.ap().rearrange("(c p) -> p c", p=128))
        bk_t = singles.tile([128, MC], F32)
        nc.sync.dma_start(out=bk_t, in_=bk_d.ap().rearrange("(c p) -> p c", p=128))
        bv_row = singles.tile([1, HD], F32R)
        nc.sync.dma_start(out=bv_row, in_=bv_d.ap().rearrange("(o d) -> o d", o=1))
        bo_row = singles.tile([1, D], F32R)
        nc.sync.dma_start(out=bo_row, in_=bo_d.ap().rearrange("(o d) -> o d", o=1))
        gamma_bc = singles.tile([128, D], F32)
        nc.sync.dma_start(out=gamma_bc, in_=_bcast_rows(gamma_d.ap(), 128))
        beta_bc = singles.tile([128, D], F32)
        nc.sync.dma_start(out=beta_bc, in_=_bcast_rows(beta_d.ap(), 128))

        aux_ps = top.enter_context(
            tc.tile_pool(name="aux_ps", bufs=3, space="PSUM"))

        # Persistent activations
        xq_nat_p = top.enter_context(tc.tile_pool(name="xq_nat", bufs=NSB))
        qT_p = top.enter_context(tc.tile_pool(name="qT", bufs=MC))
        kT_p = top.enter_context(tc.tile_pool(name="kT", bufs=MC))
        v_p = top.enter_context(tc.tile_pool(name="v", bufs=JC))
        ctxT_p = top.enter_context(tc.tile_pool(name="ctxT", bufs=MC))

        xq_nat = [xq_nat_p.tile([128, D], F32R, tag="xq_nat", name="xq_nat")
                  for _ in range(NSB)]
        qT = [qT_p.tile([128, S_CORE], F32R, tag="qT", name="qT")
              for _ in range(MC)]
        kT = [kT_p.tile([128, HALO], F32R, tag="kT", name="kT")
              for _ in range(MC)]
        v_sb = [v_p.tile([128, HD], F32R, tag="v", name="v")
                for _ in range(JC)]
        ctxT = [ctxT_p.tile([128, S_CORE], F32R, tag="ctxT", name="ctxT")
                for _ in range(HT)]

        def transpose_in(x_dram, rows, xt_pool, nat_pool, keep_tiles=None):
            """Load [rows, D] natural, return X^T as DC tiles [128, rows]."""
            xt = [xt_pool.tile([128, rows], F32R, tag="xt", name="xt")
                  for _ in range(DC)]
            for rr in range(rows // 128):
                if keep_tiles is not None:
                    nat = keep_tiles[rr]
                else:
                    nat = nat_pool.tile([128, D], F32R, tag="nat", name="nat")
                nc.sync.dma_start(out=nat,
                                  in_=x_dram.ap()[rr * 128:(rr + 1) * 128, :])
                for c in range(DC):
                    pt = aux_ps.tile([128, 128], F32R, tag="aux", name="aux")
                    nc.tensor.transpose(pt, nat[:, c * 128:(c + 1) * 128],
                                        ident)
                    nc.scalar.copy(xt[c][:, rr * 128:(rr + 1) * 128], pt)
            return xt

        # ---- Phase P: projections (k, v, then q) ----
        with ExitStack() as pp:
            big_ps = pp.enter_context(
                tc.tile_pool(name="big_ps", bufs=2, space="PSUM"))
            nat_p = pp.enter_context(tc.tile_pool(name="nat", bufs=3))

            # k^T
            with ExitStack() as ps_scope:
                xt_p = ps_scope.enter_context(tc.tile_pool(name="xkt", bufs=DC))
                w_p = ps_scope.enter_context(tc.tile_pool(name="wk", bufs=DC))
                xkT = transpose_in(xk_d, HALO, xt_p, nat_p)
                wk_sb = [w_p.tile([128, HD], F32R, tag="w", name="w")
                         for _ in range(DC)]
                for c in range(DC):
                    nc.sync.dma_start(out=wk_sb[c],
                                      in_=wk_d.ap()[c * 128:(c + 1) * 128, :])
                for m in range(MC):
                    for hf in range(HALO // 512):
                        ps_t = big_ps.tile([128, 512], F32, tag="big",
                                           name="big")
                        for kc in range(DC):
                            nc.tensor.matmul(
                                ps_t,
                                lhsT=wk_sb[kc][:, m * 128:(m + 1) * 128],
                                rhs=xkT[kc][:, hf * 512:(hf + 1) * 512],
                                start=(kc == 0), stop=(kc == DC - 1))
                        nc.scalar.activation(
                            kT[m][:, hf * 512:(hf + 1) * 512], ps_t,
                            mybir.ActivationFunctionType.Identity,
                            bias=bk_t[:, m:m + 1], scale=1.0)

            # v
            with ExitStack() as ps_scope:
                xt_p = ps_scope.enter_context(tc.tile_pool(name="xvt", bufs=DC))
                w_p = ps_scope.enter_context(tc.tile_pool(name="wv", bufs=DC))
                xvT = transpose_in(xv_d, HALO, xt_p, nat_p)
                wv_sb = [w_p.tile([128, HD], F32R, tag="w", name="w")
                         for _ in range(DC)]
                for c in range(DC):
                    nc.sync.dma_start(out=wv_sb[c],
                                      in_=wv_d.ap()[c * 128:(c + 1) * 128, :])
                for jr in range(JC):
                    for hf in range(HD // 512):
                        ps_t = big_ps.tile([128, 512], F32, tag="big",
                                           name="big")
                        for kc in range(DC):
                            nc.tensor.matmul(
                                ps_t,
                                lhsT=xvT[kc][:, jr * 128:(jr + 1) * 128],
                                rhs=wv_sb[kc][:, hf * 512:(hf + 1) * 512],
                                start=(kc == 0), stop=False)
                        nc.tensor.matmul(
                            ps_t, lhsT=ones1,
                            rhs=bv_row[:, hf * 512:(hf + 1) * 512],
                            start=False, stop=True)
                        nc.scalar.copy(v_sb[jr][:, hf * 512:(hf + 1) * 512],
                                       ps_t)

            # q^T (pre-scaled by 1/sqrt(DK); bias bq8 = bq/sqrt(DK) host-side)
            with ExitStack() as ps_scope:
                xt_p = ps_scope.enter_context(tc.tile_pool(name="xqt", bufs=DC))
                w_p = ps_scope.enter_context(tc.tile_pool(name="wq", bufs=DC))
                xqT = transpose_in(xq_d, S_CORE, xt_p, nat_p,
                                   keep_tiles=xq_nat)
                wq_sb = [w_p.tile([128, HD], F32R, tag="w", name="w")
                         for _ in range(DC)]
                for c in range(DC):
                    nc.sync.dma_start(out=wq_sb[c],
                                      in_=wq_d.ap()[c * 128:(c + 1) * 128, :])
                for m in range(MC):
                    ps_t = big_ps.tile([128, S_CORE], F32, tag="big",
                                       name="big")
                    for kc in range(DC):
                        nc.tensor.matmul(
                            ps_t,
                            lhsT=wq_sb[kc][:, m * 128:(m + 1) * 128],
                            rhs=xqT[kc],
                            start=(kc == 0), stop=(kc == DC - 1))
                    nc.scalar.activation(
                        qT[m], ps_t,
                        mybir.ActivationFunctionType.Identity,
                        bias=bq8_t[:, m:m + 1], scale=1.0 / math.sqrt(DK))

        # ---- Phase A: banded attention ----
        with ExitStack() as pa:
            score_ps = pa.enter_context(
                tc.tile_pool(name="score_ps", bufs=2, space="PSUM"))
            mask_p = pa.enter_context(tc.tile_pool(name="mask", bufs=2 * NQB))
            exp_p = pa.enter_context(tc.tile_pool(name="exp", bufs=3))
            at_p = pa.enter_context(tc.tile_pool(name="at", bufs=4))
            expT_p = pa.enter_context(
                tc.tile_pool(name="expT", bufs=2 * SJC + 4))
            den_p = pa.enter_context(tc.tile_pool(name="den", bufs=6))

            mask_sb = [[mask_p.tile([128, STRIPE], F32R, tag="mask",
                                    name="mask")
                        for _ in range(2)] for _ in range(NQB)]
            for t in range(NQB):
                for o in range(2):
                    nc.sync.dma_start(out=mask_sb[t][o],
                                      in_=mask_d.ap()[t, o])

            for mt in range(HT):
                for t in range(NQB):
                    j0 = t * 256
                    pair_expT = []
                    for hh in range(2):
                        h = 2 * mt + hh
                        po = hh * DK
                        expT = [expT_p.tile([128, 256], F32R, tag="expT",
                                            name="expT")
                                for _ in range(SJC)]
                        pair_expT.append(expT)
                        # zero the two fully-masked corner halves the
                        # transposes below skip
                        nc.sync.dma_start(out=expT[0][:, 128:256],
                                          in_=zeros_d.ap())
                        nc.sync.dma_start(out=expT[SJC - 1][:, 0:128],
                                          in_=zeros_d.ap())
                        for o in range(2):
                            i_off = t * 256 + o * 128
                            # in-band stripe slice: corner 128 cols on one
                            # side are fully masked -> skip exp/attn there
                            sl = slice(0, 640) if o == 0 else slice(128, 768)
                            jcs = range(0, SJC - 1) if o == 0 else range(1, SJC)
                            ps_s = score_ps.tile([128, STRIPE], F32,
                                                 tag="score", name="score")
                            lhs = qT[mt][po:po + DK, i_off:i_off + 128]
                            nc.tensor.matmul(
                                ps_s[:, 0:512], lhsT=lhs,
                                rhs=kT[mt][po:po + DK, j0:j0 + 512],
                                start=True, stop=False,
                                skip_group_check=True)
                            nc.tensor.matmul(
                                ps_s[:, 0:256], lhsT=ident,
                                rhs=mask_sb[t][o][:, 0:256],
                                start=False, stop=True,
                                skip_group_check=True)
                            nc.tensor.matmul(
                                ps_s[:, 512:STRIPE], lhsT=lhs,
                                rhs=kT[mt][po:po + DK, j0 + 512:j0 + STRIPE],
                                start=True, stop=False)
                            nc.tensor.matmul(
                                ps_s[:, 512:STRIPE], lhsT=ident,
                                rhs=mask_sb[t][o][:, 512:STRIPE],
                                start=False, stop=True)
                            exp_t = exp_p.tile([128, STRIPE], F32, tag="exp",
                                               name="exp")
                            den = den_p.tile([128, 1], F32, tag="den",
                                             name="den")
                            nc.scalar.activation(
                                exp_t[:, sl], ps_s[:, sl],
                                mybir.ActivationFunctionType.Exp,
                                accum_out=den)
                            rec = den_p.tile([128, 1], F32, tag="rec",
                                             name="rec")
                            nc.vector.reciprocal(rec, den)
                            at = at_p.tile([128, STRIPE], F32R, tag="at",
                                           name="at")
                            nc.vector.tensor_scalar_mul(at[:, sl],
                                                        exp_t[:, sl], rec)
                            nc.sync.dma_start(out=attn_d.ap()[h, t, o][:, sl],
                                              in_=at[:, sl])
                            for jc in jcs:
                                pt = aux_ps.tile([128, 128], F32R, tag="aux",
                                                 name="aux")
                                nc.tensor.transpose(
                                    pt, at[:, jc * 128:(jc + 1) * 128], ident)
                                nc.vector.tensor_copy(
                                    expT[jc][:, o * 128:(o + 1) * 128], pt)
                    # ctx per head; odd head lands in ctxT partitions 64:128
                    # via an SBUF->SBUF DMA (engines can't shift partitions)
                    for hh in range(2):
                        h = 2 * mt + hh
                        ps_c = aux_ps.tile([64, 256], F32, tag="aux",
                                           name="aux")
                        for jc in range(SJC):
                            jr = 2 * t + jc
                            nc.tensor.matmul(
                                ps_c,
                                lhsT=v_sb[jr][:, h * DV:(h + 1) * DV],
                                rhs=pair_expT[hh][jc],
                                start=(jc == 0), stop=(jc == SJC - 1))
                        if hh == 0:
                            nc.scalar.copy(
                                ctxT[mt][0:64, t * 256:(t + 1) * 256], ps_c)
                        else:
                            stage = at_p.tile([64, 256], F32R, tag="stage",
                                              name="stage")
                            nc.scalar.copy(stage, ps_c)
                            nc.sync.dma_start(
                                out=ctxT[mt][64:128, t * 256:(t + 1) * 256],
                                in_=stage)

        # ---- Phase O: output projection + residual + LayerNorm ----
        with ExitStack() as po_:
            big_ps = po_.enter_context(
                tc.tile_pool(name="big_ps2", bufs=2, space="PSUM"))
            wo_p = po_.enter_context(tc.tile_pool(name="wo", bufs=MC))
            x_p = po_.enter_context(tc.tile_pool(name="x", bufs=NSB))
            y_p = po_.enter_context(tc.tile_pool(name="y", bufs=2))
            st_p = po_.enter_context(tc.tile_pool(name="st", bufs=8))

            wo_sb = [wo_p.tile([128, D], F32R, tag="wo", name="wo")
                     for _ in range(MC)]
            for c in range(MC):
                nc.sync.dma_start(out=wo_sb[c],
                                  in_=wo_d.ap()[c * 128:(c + 1) * 128, :])

            for s_i in range(NSB):
                x_sb = x_p.tile([128, D], F32, tag="x", name="x")
                for hf in range(D // 512):
                    ps_t = big_ps.tile([128, 512], F32, tag="big2",
                                       name="big2")
                    for kc in range(MC):
                        nc.tensor.matmul(
                            ps_t,
                            lhsT=ctxT[kc][:, s_i * 128:(s_i + 1) * 128],
                            rhs=wo_sb[kc][:, hf * 512:(hf + 1) * 512],
                            start=(kc == 0), stop=False)
                    nc.tensor.matmul(
                        ps_t, lhsT=ones1,
                        rhs=bo_row[:, hf * 512:(hf + 1) * 512],
                        start=False, stop=True)
                    nc.scalar.copy(x_sb[:, hf * 512:(hf + 1) * 512], ps_t)
                nc.vector.tensor_add(x_sb, x_sb, xq_nat[s_i])
                # LayerNorm over D
                stats = st_p.tile([128, D // 512, 6], F32, tag="stats",
                                  name="stats")
                for g in range(D // 512):
                    nc.vector.bn_stats(stats[:, g, :],
                                       x_sb[:, g * 512:(g + 1) * 512])
                mv = st_p.tile([128, 2], F32, tag="mv", name="mv")
                nc.vector.bn_aggr(mv, stats)
                std = st_p.tile([128, 1], F32, tag="std", name="std")
                nc.scalar.activation(std, mv[:, 1:2],
                                     mybir.ActivationFunctionType.Sqrt,
                                     bias=eps_t, scale=1.0)
                rstd = st_p.tile([128, 1], F32, tag="rstd", name="rstd")
                nc.vector.reciprocal(rstd, std)
                y_sb = y_p.tile([128, D], F32, tag="y", name="y")
                nc.vector.tensor_scalar(
                    out=y_sb, in0=x_sb,
                    scalar1=mv[:, 0:1], scalar2=rstd,
                    op0=mybir.AluOpType.subtract, op1=mybir.AluOpType.mult)
                nc.vector.tensor_mul(y_sb, y_sb, gamma_bc)
                nc.vector.tensor_add(y_sb, y_sb, beta_bc)
                nc.sync.dma_start(out=y_d.ap()[s_i * 128:(s_i + 1) * 128, :],
                                  in_=y_sb)

    return nc


_PROGRAM = None


def _get_program():
    global _PROGRAM
    if _PROGRAM is None:
        _PROGRAM = build_program()
    return _PROGRAM


def _make_core_inputs(Q, K, V, Wq, bq, Wk, bk, Wv, bv, Wo, bo, gamma, beta):
    scale = 1.0 / math.sqrt(DK)
    ident_np = np.eye(128, dtype=np.float32)
    ones_np = np.ones((1, 128), np.float32)
    in_maps = []
    for c in range(N_CORES):
        b = c // SEQ_SHARDS
        q0 = (c % SEQ_SHARDS) * S_CORE
        lo = q0 - NW
        hi = q0 + S_CORE + NW
        clo, chi = max(lo, 0), min(hi, S)
        xk = np.zeros((HALO, D), np.float32)
        xv = np.zeros((HALO, D), np.float32)
        xk[clo - lo:chi - lo] = K[b, clo:chi]
        xv[clo - lo:chi - lo] = V[b, clo:chi]
        mask = np.full((NQB, 2, 128, STRIPE), MASK_NEG, np.float32)
        for t in range(NQB):
            for o in range(2):
                i_idx = q0 + t * 256 + o * 128 + np.arange(128)[:, None]
                j_idx = q0 + t * 256 - NW + np.arange(STRIPE)[None, :]
                ok = (np.abs(i_idx - j_idx) < NW) & (j_idx >= 0) & (j_idx < S)
                mask[t, o][ok] = 0.0
        in_maps.append({
            "xq": np.ascontiguousarray(Q[b, q0:q0 + S_CORE]).astype(np.float32),
            "xk": xk, "xv": xv,
            "wq": np.asarray(Wq, np.float32), "wk": np.asarray(Wk, np.float32),
            "wv": np.asarray(Wv, np.float32), "wo": np.asarray(Wo, np.float32),
            "bq8": (np.asarray(bq, np.float32) * scale).astype(np.float32),
            "bk": np.asarray(bk, np.float32), "bv": np.asarray(bv, np.float32),
            "bo": np.asarray(bo, np.float32),
            "gamma": np.asarray(gamma, np.float32),
            "beta": np.asarray(beta, np.float32),
            "mask": mask,
            "ident": ident_np, "ones1": ones_np,
            "zeros128": np.zeros((128, 128), np.float32),
        })
    return in_maps


def _assemble(results):
    y = np.zeros((B, S, D), np.float32)
    attn = np.zeros((B, H, S, S), np.float32)
    for c in range(N_CORES):
        b = c // SEQ_SHARDS
        q0 = (c % SEQ_SHARDS) * S_CORE
        y[b, q0:q0 + S_CORE] = results[c]["y"]
        stripes = np.asarray(results[c]["attn"])  # [H, NQB, 2, 128, STRIPE]
        for t in range(NQB):
            jlo = q0 + t * 256 - NW
            a, bnd = max(jlo, 0), min(jlo + STRIPE, S)
            rows = stripes[:, t].reshape(H, 256, STRIPE)
            i0 = q0 + t * 256
            attn[b, :, i0:i0 + 256, a:bnd] = rows[:, :, a - jlo:bnd - jlo]
    return y, attn


def kernel(Q, K, V, Wq, bq, Wk, bk, Wv, bv, Wo, bo, gamma, beta):
    from concourse.bass_utils import run_bass_kernel_spmd

    nc = _get_program()
    in_maps = _make_core_inputs(Q, K, V, Wq, bq, Wk, bk, Wv, bv, Wo, bo,
                                gamma, beta)
    res = run_bass_kernel_spmd(nc, in_maps, list(range(N_CORES)))
    return _assemble(res.results)
